# revision 32
# baseline (speedup 1.0000x reference)
"""LucidLinearAttention Trainium2 kernel v4 (8-core SPMD, fp8 DoubleRow).

Sharding: batch b = core//2 (4 batches), head-group hg = core%2 (8 heads each).

v4: the four big projections (Q/K/V in, Y out) run as fp8e4 DoubleRow
matmuls (2 K-chunks folded per instruction, 0.5 cycles/row) with a staged
per-block precision schedule. Weights are pre-scaled by 64 on the host so
their N(0, 1/1024) entries land in e4m3's normal range; the 1/64 is undone
by the activation scale (Q/K: exp(psum/64); V: copy*(1/64)) and on the host
for y. Error-compensation terms (hi+lo residual matmuls) are added for
early blocks, where few-key averaging amplifies quantization noise:
  Q: comp3 block 0, pure fp8 after      K,V: comp3, comp_w x2, then fp8
  Y: comp3 blocks 0-1, comp_w after (Wo residual is host-side and free)
comp_w = one extra DoubleRow pass vs fp8; comp3 = two.

Attention part (bf16, unchanged math): per 128-token chunk, C tiles are
built by two matmuls (full-chunk K=128 pcc + first-bucket K=64 cb) into
[64,132] psum slots grouped 3-3-2 per chunk so the mids adds / cb copies /
mids_b casts run as 12 batched ops per block instead of 100+ tiny ones.
"""
import sys
import numpy as np

for p in ("/opt/trn_rl_repo", "/root/.axon_site/_ro/trn_rl_repo"):
    if p not in sys.path:
        sys.path.insert(0, p)

import concourse.mybir as mybir
import concourse.tile as tile
from concourse import bacc
from concourse.bass_utils import run_bass_kernel_spmd

F32 = mybir.dt.float32
BF16 = mybir.dt.bfloat16
F8 = mybir.dt.float8e4
EXP = mybir.ActivationFunctionType.Exp
COPY = mybir.ActivationFunctionType.Copy
DR = mybir.MatmulPerfMode.DoubleRow

B, T, D = 4, 4096, 1024
NH, HD, BUCKET = 16, 64, 64
HPC = 8            # heads per core
GD = HPC * HD      # 512 group dim
NBLK = 8           # coarse blocks
BT = T // NBLK     # 512 rows per block
NCH = BT // 128    # 4 chunks of 128 per block
NC_CORES = 8
EPS = 1e-6         # kcum seed; keeps den > 0
SW = 64.0          # host weight pre-scale

# per-block precision modes: f8 = 1 pass, cw = +W-residual, c3 = +x-residual
QM = ["cw", "f8", "f8", "f8", "f8", "f8", "f8", "f8"]
KM = ["c3", "cw", "f8", "f8", "f8", "f8", "f8", "f8"]
VM = ["c3", "cw", "f8", "f8", "f8", "f8", "f8", "f8"]
YM = ["c3", "c3", "cw", "cw", "f8", "f8", "f8", "f8"]

GROUPS = [(0, 3), (3, 6), (6, 8)]  # head groups per smalls psum bank

_CACHE = {}


def _build():
    nc = bacc.Bacc("TRN2", target_bir_lowering=False, debug=False,
                   num_devices=NC_CORES)
    x8d = nc.dram_tensor("x8", [D, T], F8, kind="ExternalInput").ap()
    xl0d = nc.dram_tensor("xl0", [D, BT], F8, kind="ExternalInput").ap()
    wq = nc.dram_tensor("wq", [D, GD], F8, kind="ExternalInput").ap()
    wk = nc.dram_tensor("wk", [D, GD], F8, kind="ExternalInput").ap()
    wv = nc.dram_tensor("wv", [D, GD], F8, kind="ExternalInput").ap()
    wql = nc.dram_tensor("wql", [D, GD], F8, kind="ExternalInput").ap()
    wkl = nc.dram_tensor("wkl", [D, GD], F8, kind="ExternalInput").ap()
    wvl = nc.dram_tensor("wvl", [D, GD], F8, kind="ExternalInput").ap()
    wo = nc.dram_tensor("wo", [GD, D], F8, kind="ExternalInput").ap()
    wol = nc.dram_tensor("wol", [GD, D], F8, kind="ExternalInput").ap()
    y = nc.dram_tensor("y", [T, D], BF16, kind="ExternalOutput").ap()

    with tile.TileContext(nc) as tc:
        with nc.allow_low_precision(reason="fp8/bf16 matmul by design"), \
             tc.tile_pool(name="w", bufs=1) as wp, \
             tc.tile_pool(name="per", bufs=1) as pp, \
             tc.tile_pool(name="sb", bufs=1) as sbp, \
             tc.tile_pool(name="ps", bufs=1, space="PSUM") as ps:

            # ---- resident fp8 weights, [128, 8*GD] (d-chunk dc at GD*dc) --
            wq_t = wp.tile([128, 8 * GD], F8, tag="wq")
            wk_t = wp.tile([128, 8 * GD], F8, tag="wk")
            wv_t = wp.tile([128, 8 * GD], F8, tag="wv")
            wql_t = wp.tile([128, 8 * GD], F8, tag="wql")
            wkl_t = wp.tile([128, 8 * GD], F8, tag="wkl")
            wvl_t = wp.tile([128, 8 * GD], F8, tag="wvl")
            wo_t = wp.tile([128, 4 * D], F8, tag="wo")
            wol_t = wp.tile([128, 4 * D], F8, tag="wol")
            # slot views [128, 8, GD] / [128, 4, D]
            wq3 = wq_t[:].rearrange("k (a g) -> k a g", g=GD)
            wk3 = wk_t[:].rearrange("k (a g) -> k a g", g=GD)
            wv3 = wv_t[:].rearrange("k (a g) -> k a g", g=GD)
            wql3 = wql_t[:].rearrange("k (a g) -> k a g", g=GD)
            wkl3 = wkl_t[:].rearrange("k (a g) -> k a g", g=GD)
            wvl3 = wvl_t[:].rearrange("k (a g) -> k a g", g=GD)
            wo3 = wo_t[:].rearrange("k (a d) -> k a d", d=D)
            wol3 = wol_t[:].rearrange("k (a d) -> k a d", d=D)

            x8_0 = sbp.tile([128, 8 * BT], F8, tag="x8", name="x8", bufs=2)
            xl0_t = wp.tile([128, 8 * BT], F8, tag="xl0")

            # startup: interleave first wq/x8 chunks so Q proj starts ASAP
            wqd = wq[:].rearrange("(a p) g -> p a g", p=128)
            x3d0 = x8d[:, 0:BT].rearrange("(a p) t -> p a t", p=128)
            x30 = x8_0[:].rearrange("k (a t) -> k a t", t=BT)
            for qq in range(4):
                nc.scalar.dma_start(wq3[:, 2 * qq:2 * qq + 2, :], wqd[:, 2 * qq:2 * qq + 2, :])
                nc.sync.dma_start(x30[:, 2 * qq:2 * qq + 2, :], x3d0[:, 2 * qq:2 * qq + 2, :])
            nc.sync.dma_start(
                xl0_t[:].rearrange("k (a t) -> k a t", t=BT),
                xl0d[:].rearrange("(a p) t -> p a t", p=128))
            for wt, wd in ((wql_t, wql), (wk_t, wk), (wkl_t, wkl),
                           (wv_t, wv), (wvl_t, wvl)):
                nc.scalar.dma_start(
                    wt[:].rearrange("k (a g) -> k a g", g=GD),
                    wd[:].rearrange("(a p) g -> p a g", p=128))
            for wt, wd in ((wo_t, wo), (wol_t, wol)):
                nc.scalar.dma_start(
                    wt[:].rearrange("k (a d) -> k a d", d=D),
                    wd[:].rearrange("(a p) d -> p a d", p=128))
            xl03 = xl0_t[:].rearrange("k (a t) -> k a t", t=BT)

            # ---- persistent state --------------------------------------
            # qtu[par][p]: [128, BT] exp(q) head PAIR (2p rows 0:64, 2p+1
            # rows 64:128); odd heads' OUT matmuls run at partition base 64
            # against _hi copies of the C state
            qtu = [[pp.tile([128, BT], BF16, tag=f"qtu{s}_{p}", name=f"qtu{s}_{p}")
                    for p in range(4)] for s in range(2)]
            # per-head 66-col groups: cols 0:64 C (d x e), col 64 kcum, 65 zero
            caug_f = pp.tile([64, 8 * 66], F32, tag="caugf", name="caugf")
            caug_b = pp.tile([64, 8 * 66], BF16, tag="caugb", name="caugb")
            mids_f = [pp.tile([64, 8 * 66], F32, tag=f"midf{k}", name=f"midf{k}")
                      for k in range(3)]
            mids_b = [pp.tile([64, 8 * 66], BF16, tag=f"midb{k}", name=f"midb{k}")
                      for k in range(3)]
            # _hi twins: odd-head slices shifted to partitions 64:128 so the
            # odd-head OUT matmuls can use qtu pair rows 64:128 (Pool copies)
            caug_bh = pp.tile([128, 8 * 66], BF16, tag="caugbh", name="caugbh")
            mids_bh = [pp.tile([128, 8 * 66], BF16, tag=f"midbh{k}", name=f"midbh{k}")
                       for k in range(3)]
            cbsb_h = [pp.tile([128, 8 * 66], BF16, tag=f"cbsbh{c}", name=f"cbsbh{c}")
                      for c in range(NCH)]

            def hi_copy(dst, src):
                # odd-head 66-col groups, partitions 0:64 -> 64:128 (Pool)
                nc.gpsimd.tensor_copy(
                    dst[64:128, :].rearrange("p (h c) -> p h c", c=132)[:, :, 66:132],
                    src[:].rearrange("p (h c) -> p h c", c=132)[:, :, 66:132])
            for t_ in (caug_f, caug_b):
                nc.vector.memset(t_[:], 0.0)
                nc.vector.memset(
                    t_[:].rearrange("p (h c) -> p h c", c=66)[:, :, 64:65], EPS)
            hi_copy(caug_bh, caug_b)
            # cbsb[c]: first-bucket C tiles for the 8 heads, [64, 8*66] bf16
            cbsb = [pp.tile([64, 8 * 66], BF16, tag=f"cbsb{c}", name=f"cbsb{c}")
                    for c in range(NCH)]
            # per-pair dv: bf16 reciprocal of the broadcast-shaped den psum
            dv = [pp.tile([128, BT], BF16, tag=f"dv{p}", name=f"dv{p}") for p in range(4)]
            # vaug[par][t4]: [128, 8*66] bf16; per head: V (64) | ones | zero
            vaug = [[pp.tile([128, HPC * 66], BF16, tag=f"vaug{s}_{t}", name=f"vaug{s}_{t}")
                     for t in range(NCH)] for s in range(2)]
            for s in range(2):
                for t4 in range(NCH):
                    vv = vaug[s][t4][:].rearrange("p (h c) -> p h c", c=66)
                    nc.vector.memset(vv[:, :, 64:65], 1.0)
                    nc.vector.memset(vv[:, :, 65:66], 0.0)
            # xot fp8 [128, 4*BT]: gd-chunk p at BT*p; hi for all blocks,
            # lo only for Y=c3 blocks; xotb = bf16 staging for c3 blocks
            xot8 = [pp.tile([128, 4 * BT], F8, tag=f"xot8_{s}", name=f"xot8_{s}")
                    for s in range(2)]
            xot8l = [pp.tile([128, 4 * BT], F8, tag=f"xot8l_{s}", name=f"xot8l_{s}")
                     for s in range(2)]
            xotb = pp.tile([128, 4 * BT], BF16, tag="xotb", name="xotb")

            # ---- emit of the previous block's y ------------------------
            # ysb engine: early blocks are PE-bound (comp passes) so Act has
            # slack; late blocks are Act-bound so DVE takes the copies.
            def emit_y(t0, par_p, ymode, t4s, eng="mix"):
                xh3 = xot8[par_p][:].rearrange("k (a t) -> k a t", t=BT)
                xl3 = xot8l[par_p][:].rearrange("k (a t) -> k a t", t=BT)
                terms = [(xh3, wo3)]
                if ymode in ("cw", "c3"):
                    terms.append((xh3, wol3))
                if ymode == "c3":
                    terms.append((xl3, wo3))
                n_ins = 2 * len(terms)
                for t4 in t4s:
                    ysb = sbp.tile([128, D], BF16, tag="ysb", name="ysb", bufs=3)
                    for fc in range(2):
                        py = ps.tile([128, GD], F32, tag="po", name="py", bufs=3)
                        i = 0
                        for xt, wt in terms:
                            for jp in range(2):
                                nc.tensor.matmul(
                                    py[:],
                                    xt[:, 2 * jp:2 * jp + 2, 128 * t4:128 * (t4 + 1)],
                                    wt[:, 2 * jp:2 * jp + 2, GD * fc:GD * (fc + 1)],
                                    start=(i == 0), stop=(i == n_ins - 1),
                                    perf_mode=DR)
                                i += 1
                        on_act = (eng == "act") or (eng == "mix" and fc == 0)
                        if on_act:
                            nc.scalar.activation(
                                ysb[:, GD * fc:GD * (fc + 1)], py[:], COPY,
                                scale=1.0 / SW)
                        else:
                            nc.vector.tensor_scalar_mul(
                                ysb[:, GD * fc:GD * (fc + 1)], py[:], 1.0 / SW)
                    nc.sync.dma_start(
                        y[t0 + 128 * t4:t0 + 128 * (t4 + 1), :], ysb[:])

            prev_y = None
            x8 = x8_0
            x3 = x30
            for ct in range(NBLK):
                t0 = ct * BT
                par = ct % 2
                if ct + 1 < NBLK:
                    x8_n = sbp.tile([128, 8 * BT], F8, tag="x8", name="x8", bufs=2)
                    x3_n = x8_n[:].rearrange("k (a t) -> k a t", t=BT)
                    nc.sync.dma_start(
                        x3_n,
                        x8d[:, t0 + BT:t0 + 2 * BT].rearrange("(a p) t -> p a t", p=128))

                # ---- Q^T projection per head-pair (M=128) + exp --------
                # emits of the previous block's y are interleaved into the
                # Q/K phases, whose pace is set by Act (exp); the emit
                # matmuls keep PE busy meanwhile.
                def emit_prev(t4s):
                    if prev_y is None:
                        return
                    pt0, ppar, pym, pct = prev_y
                    emit_y(pt0, ppar, pym, t4s, "mix")

                qterms = [(wq3, x3)]
                if QM[ct] in ("cw", "c3"):
                    qterms.append((wql3, x3))
                if QM[ct] == "c3":
                    qterms.append((wq3, xl03))
                nq = 4 * len(qterms)
                for p in range(4):
                    pq = ps.tile([128, BT], F32, tag="big", name="pq", bufs=2)
                    i = 0
                    for wt, xt in qterms:
                        for j in range(4):
                            nc.tensor.matmul(
                                pq[:],
                                wt[:, 2 * j:2 * j + 2, 128 * p:128 * (p + 1)],
                                xt[:, 2 * j:2 * j + 2, :],
                                start=(i == 0), stop=(i == nq - 1), perf_mode=DR)
                            i += 1
                    nc.scalar.activation(qtu[par][p][:], pq[:], EXP,
                                         scale=1.0 / SW)
                    if p % 2 == 1:
                        emit_prev([p // 2])

                # ---- K projection per t-chunk (M=128 tokens) + exp -----
                kterms = [(x3, wk3)]
                if KM[ct] in ("cw", "c3"):
                    kterms.append((x3, wkl3))
                if KM[ct] == "c3":
                    kterms.append((xl03, wk3))
                nk = 4 * len(kterms)
                ksb = [sbp.tile([128, GD], BF16, tag=f"ksb{t}", name=f"ksb{t}", bufs=2)
                       for t in range(NCH)]
                for t4 in range(NCH):
                    pk = ps.tile([128, GD], F32, tag="big", name="pk", bufs=2)
                    i = 0
                    for xt, wt in kterms:
                        for j in range(4):
                            nc.tensor.matmul(
                                pk[:],
                                xt[:, 2 * j:2 * j + 2, 128 * t4:128 * (t4 + 1)],
                                wt[:, 2 * j:2 * j + 2, :],
                                start=(i == 0), stop=(i == nk - 1), perf_mode=DR)
                            i += 1
                    nc.scalar.activation(ksb[t4][:], pk[:], EXP, scale=1.0 / SW)
                    if t4 % 2 == 1:
                        emit_prev([2 + t4 // 2])

                # ---- V projection + smalls, interleaved ----------------
                vterms = [(x3, wv3)]
                if VM[ct] in ("cw", "c3"):
                    vterms.append((x3, wvl3))
                if VM[ct] == "c3":
                    vterms.append((xl03, wv3))
                nv = 4 * len(vterms)

                def vproj(t4):
                    pv = ps.tile([128, GD], F32, tag="big", name="pv", bufs=2)
                    i = 0
                    for xt, wt in vterms:
                        for j in range(4):
                            nc.tensor.matmul(
                                pv[:],
                                xt[:, 2 * j:2 * j + 2, 128 * t4:128 * (t4 + 1)],
                                wt[:, 2 * j:2 * j + 2, :],
                                start=(i == 0), stop=(i == nv - 1), perf_mode=DR)
                            i += 1
                    vv = vaug[par][t4][:].rearrange("p (h c) -> p h c", c=66)
                    pvv = pv[:].rearrange("p (h c) -> p h c", c=64)
                    nc.scalar.activation(vv[:, :, 0:64], pvv[:, :, :], COPY,
                                         scale=1.0 / SW)

                def smalls(c):
                    # per head: pcc (K=128) and cb (K=64) into a [64,132] slot;
                    # slots grouped [3,3,2] per psum bank for batched post-ops
                    for h0, h1 in GROUPS:
                        ng = h1 - h0
                        sm = ps.tile([64, 132 * ng], F32, tag="sm", name="sm",
                                     bufs=3, padded_shape=[128, 512])
                        for hh in range(h0, h1):
                            o = 132 * (hh - h0)
                            va = vaug[par][c][:, 66 * hh:66 * (hh + 1)]
                            nc.tensor.matmul(
                                sm[:, o:o + 66], ksb[c][:, 64 * hh:64 * (hh + 1)],
                                va, start=True, stop=True)
                            nc.tensor.matmul(
                                sm[:, o + 66:o + 132],
                                ksb[c][0:64, 64 * hh:64 * (hh + 1)],
                                va[0:64, :], start=True, stop=True)
                        sm3 = sm[:].rearrange("p (g c) -> p g c", c=132)
                        cb3 = cbsb[c][:, 66 * h0:66 * h1].rearrange(
                            "p (g c) -> p g c", c=66)
                        nc.scalar.activation(cb3, sm3[:, :, 66:132], COPY)
                        sl = slice(66 * h0, 66 * h1)
                        dst3 = "p (g c) -> p g c"
                        if c == 0:
                            nc.vector.tensor_add(
                                mids_f[0][:, sl].rearrange(dst3, c=66),
                                caug_f[:, sl].rearrange(dst3, c=66),
                                sm3[:, :, 0:66])
                        elif c < NCH - 1:
                            nc.vector.tensor_add(
                                mids_f[c][:, sl].rearrange(dst3, c=66),
                                mids_f[c - 1][:, sl].rearrange(dst3, c=66),
                                sm3[:, :, 0:66])
                        else:
                            nc.vector.tensor_add(
                                caug_f[:, sl].rearrange(dst3, c=66),
                                mids_f[c - 1][:, sl].rearrange(dst3, c=66),
                                sm3[:, :, 0:66])
                        if c < NCH - 1:
                            nc.gpsimd.tensor_copy(
                                mids_b[c][:, sl], mids_f[c][:, sl])
                    hi_copy(cbsb_h[c], cbsb[c])
                    if c < NCH - 1:
                        hi_copy(mids_bh[c], mids_b[c])

                vproj(0)
                vproj(1)
                smalls(0)
                vproj(2)
                smalls(1)
                vproj(3)
                smalls(2)
                smalls(3)

                # ---- OUT phase: paired [128, BT] po + per-head M=1 dens --
                po_t = [None] * 4
                for p in range(4):
                    po = ps.tile([128, BT], F32, tag="po", name="po", bufs=3)
                    po_t[p] = po
                    # sm banks are free during OUT; using them keeps the next
                    # block's pq (big pool) from waiting on the recip tail
                    pd = ps.tile([128, BT], F32, tag="sm", name="pd", bufs=3,
                                 padded_shape=[128, 512])
                    # dens first: the recip (DVE) then overlaps the po
                    # matmuls instead of trailing the whole pair. Odd heads
                    # (rows 64:128) use the _hi state copies + qtu rows
                    # 64:128 with matched partition bases.
                    qt_p = qtu[par][p]
                    for hh in range(2):
                        h = 2 * p + hh
                        r = 64 * hh
                        rs = slice(r, r + 64)
                        for c in range(NCH):
                            c0 = 128 * c
                            if hh == 0:
                                bs = (caug_b if c == 0 else mids_b[c - 1])
                                cbs = cbsb[c]
                            else:
                                bs = (caug_bh if c == 0 else mids_bh[c - 1])
                                cbs = cbsb_h[c]
                            # den, pre-broadcast across 64 rows via stride-0 lhsT
                            nc.tensor.matmul(
                                pd[rs, c0:c0 + 128],
                                bs[rs, 66 * h + 64:66 * h + 65].broadcast_to([64, 64]),
                                qt_p[rs, c0:c0 + 128], start=True, stop=False)
                            nc.tensor.matmul(
                                pd[rs, c0 + 64:c0 + 128],
                                cbs[rs, 66 * h + 64:66 * h + 65].broadcast_to([64, 64]),
                                qt_p[rs, c0 + 64:c0 + 128], start=False, stop=True)
                    nc.vector.reciprocal(dv[p][:], pd[:])
                    for hh in range(2):
                        h = 2 * p + hh
                        r = 64 * hh
                        rs = slice(r, r + 64)
                        for c in range(NCH):
                            c0 = 128 * c
                            if hh == 0:
                                bs = (caug_b if c == 0 else mids_b[c - 1])
                                cbs = cbsb[c]
                            else:
                                bs = (caug_bh if c == 0 else mids_bh[c - 1])
                                cbs = cbsb_h[c]
                            nc.tensor.matmul(
                                po[rs, c0:c0 + 128],
                                bs[rs, 66 * h:66 * h + 64],
                                qt_p[rs, c0:c0 + 128], start=True, stop=False)
                            nc.tensor.matmul(
                                po[rs, c0 + 64:c0 + 128],
                                cbs[rs, 66 * h:66 * h + 64],
                                qt_p[rs, c0 + 64:c0 + 128], start=False, stop=True)
                    if p == 3:
                        nc.gpsimd.tensor_copy(caug_b[:], caug_f[:])
                        hi_copy(caug_bh, caug_b)
                    if p > 0:
                        _norm(nc, p - 1, po_t, dv, xot8, xot8l, xotb,
                              par, YM[ct])
                _norm(nc, 3, po_t, dv, xot8, xot8l, xotb, par, YM[ct])

                prev_y = (t0, par, YM[ct], ct)
                if ct + 1 < NBLK:
                    x8 = x8_n
                    x3 = x3_n
            emit_y(prev_y[0], prev_y[1], prev_y[2], list(range(NCH)), "mix")

    nc.compile()
    return nc


def _norm(nc, p, po_t, dv, xot8, xot8l, xotb, par, ymode):
    po = po_t[p]
    cs = slice(BT * p, BT * (p + 1))
    if ymode == "c3":
        nc.vector.tensor_mul(xotb[:, cs], po[:], dv[p][:])
        nc.gpsimd.tensor_copy(xot8[par][:, cs], xotb[:, cs])
        nc.gpsimd.tensor_sub(xot8l[par][:, cs], xotb[:, cs], xot8[par][:, cs])
    else:
        nc.vector.tensor_mul(xot8[par][:, cs], po[:], dv[p][:])


def _get_nc():
    if "nc" not in _CACHE:
        _CACHE["nc"] = _build()
    return _CACHE["nc"]


def kernel(x, W_qkv, W_out):
    import ml_dtypes
    f8 = ml_dtypes.float8_e4m3
    x = np.asarray(x, dtype=np.float32)
    W_qkv = np.asarray(W_qkv, dtype=np.float32)
    W_out = np.asarray(W_out, dtype=np.float32)
    nc = _get_nc()

    def split8(a):
        hi = a.astype(f8)
        lo = (a - hi.astype(np.float32)).astype(f8)
        return hi, lo

    x8s, xl0s = [], []
    for b in range(B):
        xT = np.ascontiguousarray(x[b].T)          # [D, T]
        hi, lo = split8(xT)
        x8s.append(hi)
        xl0s.append(np.ascontiguousarray(lo[:, 0:BT]))

    in_maps = []
    for c in range(NC_CORES):
        b, hg = c // 2, c % 2
        s = slice(hg * GD, (hg + 1) * GD)
        wqh, wql_ = split8(np.ascontiguousarray(W_qkv[0 * D:1 * D][s].T) * SW)
        wkh, wkl_ = split8(np.ascontiguousarray(W_qkv[1 * D:2 * D][s].T) * SW)
        wvh, wvl_ = split8(np.ascontiguousarray(W_qkv[2 * D:3 * D][s].T) * SW)
        woh, wol_ = split8(np.ascontiguousarray(W_out[:, s].T) * SW)
        in_maps.append({
            "x8": x8s[b], "xl0": xl0s[b],
            "wq": wqh, "wql": wql_,
            "wk": wkh, "wkl": wkl_,
            "wv": wvh, "wvl": wvl_,
            "wo": woh, "wol": wol_,
        })
    res = run_bass_kernel_spmd(nc, in_maps, core_ids=list(range(NC_CORES)))
    out = np.empty((B, T, D), dtype=np.float32)
    for b in range(B):
        out[b] = (res.results[2 * b]["y"].astype(np.float32)
                  + res.results[2 * b + 1]["y"].astype(np.float32))
    return out


# revision 57
# speedup vs baseline: 1.0918x; 1.0918x over previous
"""LucidLinearAttention Trainium2 kernel v4 (8-core SPMD, fp8 DoubleRow).

Sharding: batch b = core//2 (4 batches), head-group hg = core%2 (8 heads each).

v4: the four big projections (Q/K/V in, Y out) run as fp8e4 DoubleRow
matmuls (2 K-chunks folded per instruction, 0.5 cycles/row) with a staged
per-block precision schedule. Weights are pre-scaled by 64 on the host so
their N(0, 1/1024) entries land in e4m3's normal range; the 1/64 is undone
by the activation scale (Q/K: exp(psum/64); V: copy*(1/64)) and on the host
for y. Error-compensation terms (hi+lo residual matmuls) are added for
early blocks, where few-key averaging amplifies quantization noise:
  Q: comp3 block 0, pure fp8 after      K,V: comp3, comp_w x2, then fp8
  Y: comp3 blocks 0-1, comp_w after (Wo residual is host-side and free)
comp_w = one extra DoubleRow pass vs fp8; comp3 = two.

Attention part (bf16, unchanged math): per 128-token chunk, C tiles are
built by two matmuls (full-chunk K=128 pcc + first-bucket K=64 cb) into
[64,132] psum slots grouped 3-3-2 per chunk so the mids adds / cb copies /
mids_b casts run as 12 batched ops per block instead of 100+ tiny ones.
"""
import sys
import numpy as np

for p in ("/opt/trn_rl_repo", "/root/.axon_site/_ro/trn_rl_repo"):
    if p not in sys.path:
        sys.path.insert(0, p)

import concourse.mybir as mybir
import concourse.tile as tile
from concourse import bacc
from concourse.bass_utils import run_bass_kernel_spmd

F32 = mybir.dt.float32
BF16 = mybir.dt.bfloat16
F8 = mybir.dt.float8e4
EXP = mybir.ActivationFunctionType.Exp
COPY = mybir.ActivationFunctionType.Copy
DR = mybir.MatmulPerfMode.DoubleRow

B, T, D = 4, 4096, 1024
NH, HD, BUCKET = 16, 64, 64
HPC = 8            # heads per core
GD = HPC * HD      # 512 group dim
NBLK = 8           # coarse blocks
BT = T // NBLK     # 512 rows per block
NCH = BT // 128    # 4 chunks of 128 per block
NC_CORES = 8
EPS = 1e-6         # kcum seed; keeps den > 0
SW = 64.0          # host weight pre-scale

# per-block precision modes: f8 = 1 pass, cw = +W-residual, c3 = +x-residual
QM = ["cw", "f8", "f8", "f8", "f8", "f8", "f8", "f8"]
KM = ["c3", "cw", "f8", "f8", "f8", "f8", "f8", "f8"]
VM = ["c3", "cw", "f8", "f8", "f8", "f8", "f8", "f8"]
YM = ["c3", "c3", "cw", "cw", "f8", "f8", "f8", "f8"]

GROUPS = [(0, 3), (3, 6), (6, 8)]  # head groups per smalls psum bank

_CACHE = {}


def _build():
    nc = bacc.Bacc("TRN2", target_bir_lowering=False, debug=False,
                   num_devices=NC_CORES)
    x8d = nc.dram_tensor("x8", [D, T], F8, kind="ExternalInput").ap()
    xl0d = nc.dram_tensor("xl0", [D, BT], F8, kind="ExternalInput").ap()
    wq = nc.dram_tensor("wq", [D, GD], F8, kind="ExternalInput").ap()
    wk = nc.dram_tensor("wk", [D, GD], F8, kind="ExternalInput").ap()
    wv = nc.dram_tensor("wv", [D, GD], F8, kind="ExternalInput").ap()
    wql = nc.dram_tensor("wql", [D, GD], F8, kind="ExternalInput").ap()
    wkl = nc.dram_tensor("wkl", [D, GD], F8, kind="ExternalInput").ap()
    wvl = nc.dram_tensor("wvl", [D, GD], F8, kind="ExternalInput").ap()
    wo = nc.dram_tensor("wo", [GD, D], F8, kind="ExternalInput").ap()
    wol = nc.dram_tensor("wol", [GD, D], F8, kind="ExternalInput").ap()
    y = nc.dram_tensor("y", [T, D], BF16, kind="ExternalOutput").ap()

    with tile.TileContext(nc) as tc:
        with nc.allow_low_precision(reason="fp8/bf16 matmul by design"), \
             tc.tile_pool(name="w", bufs=1) as wp, \
             tc.tile_pool(name="per", bufs=1) as pp, \
             tc.tile_pool(name="sb", bufs=1) as sbp, \
             tc.tile_pool(name="ps", bufs=1, space="PSUM") as ps:

            # ---- resident fp8 weights, [128, 8*GD] (d-chunk dc at GD*dc) --
            wq_t = wp.tile([128, 8 * GD], F8, tag="wq")
            wk_t = wp.tile([128, 8 * GD], F8, tag="wk")
            wv_t = wp.tile([128, 8 * GD], F8, tag="wv")
            wql_t = wp.tile([128, 8 * GD], F8, tag="wql")
            wkl_t = wp.tile([128, 8 * GD], F8, tag="wkl")
            wvl_t = wp.tile([128, 8 * GD], F8, tag="wvl")
            wo_t = wp.tile([128, 4 * D], F8, tag="wo")
            wol_t = wp.tile([128, 4 * D], F8, tag="wol")
            # slot views [128, 8, GD] / [128, 4, D]
            wq3 = wq_t[:].rearrange("k (a g) -> k a g", g=GD)
            wk3 = wk_t[:].rearrange("k (a g) -> k a g", g=GD)
            wv3 = wv_t[:].rearrange("k (a g) -> k a g", g=GD)
            wql3 = wql_t[:].rearrange("k (a g) -> k a g", g=GD)
            wkl3 = wkl_t[:].rearrange("k (a g) -> k a g", g=GD)
            wvl3 = wvl_t[:].rearrange("k (a g) -> k a g", g=GD)
            wo3 = wo_t[:].rearrange("k (a d) -> k a d", d=D)
            wol3 = wol_t[:].rearrange("k (a d) -> k a d", d=D)

            x8_0 = sbp.tile([128, 8 * BT], F8, tag="x8", name="x8", bufs=2)
            xl0_t = wp.tile([128, 8 * BT], F8, tag="xl0")

            # startup: interleave first wq/x8 chunks so Q proj starts ASAP
            wqd = wq[:].rearrange("(a p) g -> p a g", p=128)
            x3d0 = x8d[:, 0:BT].rearrange("(a p) t -> p a t", p=128)
            x30 = x8_0[:].rearrange("k (a t) -> k a t", t=BT)
            for qq in range(4):
                nc.scalar.dma_start(wq3[:, 2 * qq:2 * qq + 2, :], wqd[:, 2 * qq:2 * qq + 2, :])
                nc.sync.dma_start(x30[:, 2 * qq:2 * qq + 2, :], x3d0[:, 2 * qq:2 * qq + 2, :])
            nc.scalar.dma_start(
                wql_t[:].rearrange("k (a g) -> k a g", g=GD),
                wql[:].rearrange("(a p) g -> p a g", p=128))
            nc.sync.dma_start(
                xl0_t[:].rearrange("k (a t) -> k a t", t=BT),
                xl0d[:].rearrange("(a p) t -> p a t", p=128))
            for wt, wd in ((wk_t, wk), (wkl_t, wkl),
                           (wv_t, wv), (wvl_t, wvl)):
                nc.scalar.dma_start(
                    wt[:].rearrange("k (a g) -> k a g", g=GD),
                    wd[:].rearrange("(a p) g -> p a g", p=128))
            for wt, wd in ((wo_t, wo), (wol_t, wol)):
                nc.scalar.dma_start(
                    wt[:].rearrange("k (a d) -> k a d", d=D),
                    wd[:].rearrange("(a p) d -> p a d", p=128))
            xl03 = xl0_t[:].rearrange("k (a t) -> k a t", t=BT)

            # ---- persistent state --------------------------------------
            # qtu[par][p]: [128, BT] exp(q) head PAIR (2p rows 0:64, 2p+1
            # rows 64:128); odd heads' OUT matmuls run at partition base 64
            # against _hi copies of the C state
            qtu = [[pp.tile([128, BT], BF16, tag=f"qtu{s}_{p}", name=f"qtu{s}_{p}")
                    for p in range(4)] for s in range(2)]
            # per-head 66-col groups: cols 0:64 C (d x e), col 64 kcum, 65 zero
            caug_f = pp.tile([64, 8 * 66], F32, tag="caugf", name="caugf")
            caug_b = pp.tile([64, 8 * 66], BF16, tag="caugb", name="caugb")
            mids_f = [pp.tile([64, 8 * 66], F32, tag=f"midf{k}", name=f"midf{k}")
                      for k in range(3)]
            mids_b = [pp.tile([64, 8 * 66], BF16, tag=f"midb{k}", name=f"midb{k}")
                      for k in range(3)]
            # _hi twins: odd-head slices shifted to partitions 64:128 so the
            # odd-head OUT matmuls can use qtu pair rows 64:128 (Pool copies)
            caug_bh = pp.tile([128, 8 * 66], BF16, tag="caugbh", name="caugbh")
            mids_bh = [pp.tile([128, 8 * 66], BF16, tag=f"midbh{k}", name=f"midbh{k}")
                       for k in range(3)]
            cbsb_h = [pp.tile([128, 8 * 66], BF16, tag=f"cbsbh{c}", name=f"cbsbh{c}")
                      for c in range(NCH)]

            def hi_copy(dst, src):
                # odd-head 66-col groups, partitions 0:64 -> 64:128
                nc.vector.tensor_copy(
                    dst[64:128, :].rearrange("p (h c) -> p h c", c=132)[:, :, 66:132],
                    src[:].rearrange("p (h c) -> p h c", c=132)[:, :, 66:132])
            for t_ in (caug_f, caug_b):
                nc.vector.memset(t_[:], 0.0)
                nc.vector.memset(
                    t_[:].rearrange("p (h c) -> p h c", c=66)[:, :, 64:65], EPS)
            hi_copy(caug_bh, caug_b)
            # cbsb[c]: first-bucket C tiles for the 8 heads, [64, 8*66] bf16
            cbsb = [pp.tile([64, 8 * 66], BF16, tag=f"cbsb{c}", name=f"cbsb{c}")
                    for c in range(NCH)]
            # per-pair dv: bf16 reciprocal of the broadcast-shaped den psum
            dv = [pp.tile([128, BT], BF16, tag=f"dv{p}", name=f"dv{p}") for p in range(4)]
            # vaug[par][t4]: [128, 8*66] bf16; per head: V (64) | ones | zero
            vaug = [[pp.tile([128, HPC * 66], BF16, tag=f"vaug{s}_{t}", name=f"vaug{s}_{t}")
                     for t in range(NCH)] for s in range(2)]
            for s in range(2):
                for t4 in range(NCH):
                    vv = vaug[s][t4][:].rearrange("p (h c) -> p h c", c=66)
                    nc.vector.memset(vv[:, :, 64:65], 1.0)
                    nc.vector.memset(vv[:, :, 65:66], 0.0)
            # xot fp8 [128, 4*BT]: gd-chunk p at BT*p; hi for all blocks,
            # lo only for Y=c3 blocks; xotb = bf16 staging for c3 blocks
            xot8 = [pp.tile([128, 4 * BT], F8, tag=f"xot8_{s}", name=f"xot8_{s}")
                    for s in range(2)]
            xot8l = [pp.tile([128, 4 * BT], F8, tag=f"xot8l_{s}", name=f"xot8l_{s}")
                     for s in range(2)]
            xotb = pp.tile([128, 4 * BT], BF16, tag="xotb", name="xotb")

            # ---- emit of the previous block's y ------------------------
            # ysb engine: early blocks are PE-bound (comp passes) so Act has
            # slack; late blocks are Act-bound so DVE takes the copies.
            def emit_y(t0, par_p, ymode, t4s, eng="mix"):
                xh3 = xot8[par_p][:].rearrange("k (a t) -> k a t", t=BT)
                xl3 = xot8l[par_p][:].rearrange("k (a t) -> k a t", t=BT)
                terms = [(xh3, wo3)]
                if ymode in ("cw", "c3"):
                    terms.append((xh3, wol3))
                if ymode == "c3":
                    terms.append((xl3, wo3))
                n_ins = 2 * len(terms)
                for t4 in t4s:
                    ysb = sbp.tile([128, D], BF16, tag="ysb", name="ysb", bufs=3)
                    for fc in range(2):
                        py = ps.tile([128, GD], F32, tag="po", name="py", bufs=3)
                        i = 0
                        for xt, wt in terms:
                            for jp in range(2):
                                nc.tensor.matmul(
                                    py[:],
                                    xt[:, 2 * jp:2 * jp + 2, 128 * t4:128 * (t4 + 1)],
                                    wt[:, 2 * jp:2 * jp + 2, GD * fc:GD * (fc + 1)],
                                    start=(i == 0), stop=(i == n_ins - 1),
                                    perf_mode=DR)
                                i += 1
                        on_act = (eng == "act") or (eng == "mix" and fc == 0)
                        if on_act:
                            nc.scalar.activation(
                                ysb[:, GD * fc:GD * (fc + 1)], py[:], COPY,
                                scale=1.0 / SW)
                        else:
                            nc.vector.tensor_scalar_mul(
                                ysb[:, GD * fc:GD * (fc + 1)], py[:], 1.0 / SW)
                    nc.sync.dma_start(
                        y[t0 + 128 * t4:t0 + 128 * (t4 + 1), :], ysb[:])

            prev_y = None
            x8 = x8_0
            x3 = x30
            for ct in range(NBLK):
                t0 = ct * BT
                par = ct % 2
                if ct + 1 < NBLK:
                    x8_n = sbp.tile([128, 8 * BT], F8, tag="x8", name="x8", bufs=2)
                    x3_n = x8_n[:].rearrange("k (a t) -> k a t", t=BT)
                    nc.sync.dma_start(
                        x3_n,
                        x8d[:, t0 + BT:t0 + 2 * BT].rearrange("(a p) t -> p a t", p=128))

                # ---- Q^T projection per head-pair (M=128) + exp --------
                # emits of the previous block's y are interleaved into the
                # Q/K phases, whose pace is set by Act (exp); the emit
                # matmuls keep PE busy meanwhile.
                def emit_prev(t4s, eng="dve"):
                    if prev_y is None:
                        return
                    pt0, ppar, pym, pct = prev_y
                    emit_y(pt0, ppar, pym, t4s, eng)

                qterms = [(wq3, x3)]
                if QM[ct] in ("cw", "c3"):
                    qterms.append((wql3, x3))
                if QM[ct] == "c3":
                    qterms.append((wq3, xl03))
                nq = 4 * len(qterms)
                for p in range(4):
                    pq = ps.tile([128, BT], F32, tag="big", name="pq", bufs=2)
                    i = 0
                    for wt, xt in qterms:
                        for j in range(4):
                            nc.tensor.matmul(
                                pq[:],
                                wt[:, 2 * j:2 * j + 2, 128 * p:128 * (p + 1)],
                                xt[:, 2 * j:2 * j + 2, :],
                                start=(i == 0), stop=(i == nq - 1), perf_mode=DR)
                            i += 1
                    nc.scalar.activation(qtu[par][p][:], pq[:], EXP,
                                         scale=1.0 / SW)
                    if p % 2 == 1:
                        emit_prev([p // 2])

                # ---- K projection per t-chunk (M=128 tokens) + exp -----
                kterms = [(x3, wk3)]
                if KM[ct] in ("cw", "c3"):
                    kterms.append((x3, wkl3))
                if KM[ct] == "c3":
                    kterms.append((xl03, wk3))
                nk = 4 * len(kterms)
                ksb = [sbp.tile([128, GD], BF16, tag=f"ksb{t}", name=f"ksb{t}", bufs=2)
                       for t in range(NCH)]
                def kproj(t4):
                    pk = ps.tile([128, GD], F32, tag="big", name="pk", bufs=2)
                    i = 0
                    for xt, wt in kterms:
                        for j in range(4):
                            nc.tensor.matmul(
                                pk[:],
                                xt[:, 2 * j:2 * j + 2, 128 * t4:128 * (t4 + 1)],
                                wt[:, 2 * j:2 * j + 2, :],
                                start=(i == 0), stop=(i == nk - 1), perf_mode=DR)
                            i += 1
                    nc.scalar.activation(ksb[t4][:], pk[:], EXP, scale=1.0 / SW)

                # ---- V projection + smalls, interleaved ----------------
                vterms = [(x3, wv3)]
                if VM[ct] in ("cw", "c3"):
                    vterms.append((x3, wvl3))
                if VM[ct] == "c3":
                    vterms.append((xl03, wv3))
                nv = 4 * len(vterms)

                def vproj(t4):
                    pv = ps.tile([128, GD], F32, tag="big", name="pv", bufs=2)
                    i = 0
                    for xt, wt in vterms:
                        for j in range(4):
                            nc.tensor.matmul(
                                pv[:],
                                xt[:, 2 * j:2 * j + 2, 128 * t4:128 * (t4 + 1)],
                                wt[:, 2 * j:2 * j + 2, :],
                                start=(i == 0), stop=(i == nv - 1), perf_mode=DR)
                            i += 1
                    vv = vaug[par][t4][:].rearrange("p (h c) -> p h c", c=66)
                    pvv = pv[:].rearrange("p (h c) -> p h c", c=64)
                    nc.scalar.activation(vv[:, :, 0:64], pvv[:, :, :], COPY,
                                         scale=1.0 / SW)

                def smalls(c):
                    # per head: pcc (K=128) and cb (K=64) into a [64,132] slot;
                    # slots grouped [3,3,2] per psum bank for batched post-ops
                    for h0, h1 in GROUPS:
                        ng = h1 - h0
                        sm = ps.tile([64, 132 * ng], F32, tag="sm", name="sm",
                                     bufs=3, padded_shape=[128, 512])
                        for hh in range(h0, h1):
                            o = 132 * (hh - h0)
                            va = vaug[par][c][:, 66 * hh:66 * (hh + 1)]
                            nc.tensor.matmul(
                                sm[:, o:o + 66], ksb[c][:, 64 * hh:64 * (hh + 1)],
                                va, start=True, stop=True)
                            nc.tensor.matmul(
                                sm[:, o + 66:o + 132],
                                ksb[c][0:64, 64 * hh:64 * (hh + 1)],
                                va[0:64, :], start=True, stop=True)
                        sm3 = sm[:].rearrange("p (g c) -> p g c", c=132)
                        cb3 = cbsb[c][:, 66 * h0:66 * h1].rearrange(
                            "p (g c) -> p g c", c=66)
                        nc.scalar.activation(cb3, sm3[:, :, 66:132], COPY)
                        sl = slice(66 * h0, 66 * h1)
                        dst3 = "p (g c) -> p g c"
                        if c == 0:
                            nc.vector.tensor_add(
                                mids_f[0][:, sl].rearrange(dst3, c=66),
                                caug_f[:, sl].rearrange(dst3, c=66),
                                sm3[:, :, 0:66])
                        elif c < NCH - 1:
                            nc.vector.tensor_add(
                                mids_f[c][:, sl].rearrange(dst3, c=66),
                                mids_f[c - 1][:, sl].rearrange(dst3, c=66),
                                sm3[:, :, 0:66])
                        else:
                            nc.vector.tensor_add(
                                caug_f[:, sl].rearrange(dst3, c=66),
                                mids_f[c - 1][:, sl].rearrange(dst3, c=66),
                                sm3[:, :, 0:66])
                        if c < NCH - 1:
                            nc.gpsimd.tensor_copy(
                                mids_b[c][:, sl], mids_f[c][:, sl])
                    hi_copy(cbsb_h[c], cbsb[c])
                    if c < NCH - 1:
                        hi_copy(mids_bh[c], mids_b[c])

                # interleave K and V per chunk so Act's Vcopy(t4) lands
                # right after Kexp(t4) and smalls(t4) starts early
                kproj(0)
                vproj(0)
                kproj(1)
                vproj(1)
                smalls(0)
                kproj(2)
                vproj(2)
                emit_prev([2])
                smalls(1)
                kproj(3)
                vproj(3)
                emit_prev([3])
                smalls(2)
                smalls(3)

                # ---- OUT phase: paired [128, BT] po + per-head M=1 dens --
                po_t = [None] * 4
                for p in range(4):
                    po = ps.tile([128, BT], F32, tag="po", name="po", bufs=3)
                    po_t[p] = po
                    # sm banks are free during OUT; using them keeps the next
                    # block's pq (big pool) from waiting on the recip tail
                    pd = ps.tile([128, BT], F32, tag="sm", name="pd", bufs=3,
                                 padded_shape=[128, 512])
                    # dens first: the recip (DVE) then overlaps the po
                    # matmuls instead of trailing the whole pair. Odd heads
                    # (rows 64:128) use the _hi state copies + qtu rows
                    # 64:128 with matched partition bases.
                    qt_p = qtu[par][p]
                    for hh in range(2):
                        h = 2 * p + hh
                        r = 64 * hh
                        rs = slice(r, r + 64)
                        for c in range(NCH):
                            c0 = 128 * c
                            if hh == 0:
                                bs = (caug_b if c == 0 else mids_b[c - 1])
                                cbs = cbsb[c]
                            else:
                                bs = (caug_bh if c == 0 else mids_bh[c - 1])
                                cbs = cbsb_h[c]
                            # den, pre-broadcast across 64 rows via stride-0 lhsT
                            nc.tensor.matmul(
                                pd[rs, c0:c0 + 128],
                                bs[rs, 66 * h + 64:66 * h + 65].broadcast_to([64, 64]),
                                qt_p[rs, c0:c0 + 128], start=True, stop=False)
                            nc.tensor.matmul(
                                pd[rs, c0 + 64:c0 + 128],
                                cbs[rs, 66 * h + 64:66 * h + 65].broadcast_to([64, 64]),
                                qt_p[rs, c0 + 64:c0 + 128], start=False, stop=True)
                    # emit mul(p-1) before recip(p) so the DVE queue runs it
                    # first — emits of the next block wait on the muls
                    if p > 0:
                        _norm(nc, p - 1, po_t, dv, xot8, xot8l, xotb,
                              par, YM[ct])
                    nc.vector.reciprocal(dv[p][:], pd[:])
                    for hh in range(2):
                        h = 2 * p + hh
                        r = 64 * hh
                        rs = slice(r, r + 64)
                        for c in range(NCH):
                            c0 = 128 * c
                            if hh == 0:
                                bs = (caug_b if c == 0 else mids_b[c - 1])
                                cbs = cbsb[c]
                            else:
                                bs = (caug_bh if c == 0 else mids_bh[c - 1])
                                cbs = cbsb_h[c]
                            nc.tensor.matmul(
                                po[rs, c0:c0 + 128],
                                bs[rs, 66 * h:66 * h + 64],
                                qt_p[rs, c0:c0 + 128], start=True, stop=False)
                            nc.tensor.matmul(
                                po[rs, c0 + 64:c0 + 128],
                                cbs[rs, 66 * h:66 * h + 64],
                                qt_p[rs, c0 + 64:c0 + 128], start=False, stop=True)
                    if p == 3:
                        nc.gpsimd.tensor_copy(caug_b[:], caug_f[:])
                        hi_copy(caug_bh, caug_b)
                _norm(nc, 3, po_t, dv, xot8, xot8l, xotb, par, YM[ct])

                prev_y = (t0, par, YM[ct], ct)
                if ct + 1 < NBLK:
                    x8 = x8_n
                    x3 = x3_n
            emit_y(prev_y[0], prev_y[1], prev_y[2], list(range(NCH)), "mix")

    nc.compile()
    return nc


def _norm(nc, p, po_t, dv, xot8, xot8l, xotb, par, ymode):
    po = po_t[p]
    cs = slice(BT * p, BT * (p + 1))
    if ymode == "c3":
        nc.vector.tensor_mul(xotb[:, cs], po[:], dv[p][:])
        nc.gpsimd.tensor_copy(xot8[par][:, cs], xotb[:, cs])
        nc.gpsimd.tensor_sub(xot8l[par][:, cs], xotb[:, cs], xot8[par][:, cs])
    else:
        nc.vector.tensor_mul(xot8[par][:, cs], po[:], dv[p][:])


def _get_nc():
    if "nc" not in _CACHE:
        _CACHE["nc"] = _build()
    return _CACHE["nc"]


def kernel(x, W_qkv, W_out):
    import ml_dtypes
    f8 = ml_dtypes.float8_e4m3
    x = np.asarray(x, dtype=np.float32)
    W_qkv = np.asarray(W_qkv, dtype=np.float32)
    W_out = np.asarray(W_out, dtype=np.float32)
    nc = _get_nc()

    def split8(a):
        hi = a.astype(f8)
        lo = (a - hi.astype(np.float32)).astype(f8)
        return hi, lo

    x8s, xl0s = [], []
    for b in range(B):
        xT = np.ascontiguousarray(x[b].T)          # [D, T]
        hi, lo = split8(xT)
        x8s.append(hi)
        xl0s.append(np.ascontiguousarray(lo[:, 0:BT]))

    in_maps = []
    for c in range(NC_CORES):
        b, hg = c // 2, c % 2
        s = slice(hg * GD, (hg + 1) * GD)
        wqh, wql_ = split8(np.ascontiguousarray(W_qkv[0 * D:1 * D][s].T) * SW)
        wkh, wkl_ = split8(np.ascontiguousarray(W_qkv[1 * D:2 * D][s].T) * SW)
        wvh, wvl_ = split8(np.ascontiguousarray(W_qkv[2 * D:3 * D][s].T) * SW)
        woh, wol_ = split8(np.ascontiguousarray(W_out[:, s].T) * SW)
        in_maps.append({
            "x8": x8s[b], "xl0": xl0s[b],
            "wq": wqh, "wql": wql_,
            "wk": wkh, "wkl": wkl_,
            "wv": wvh, "wvl": wvl_,
            "wo": woh, "wol": wol_,
        })
    res = run_bass_kernel_spmd(nc, in_maps, core_ids=list(range(NC_CORES)))
    out = np.empty((B, T, D), dtype=np.float32)
    for b in range(B):
        out[b] = (res.results[2 * b]["y"].astype(np.float32)
                  + res.results[2 * b + 1]["y"].astype(np.float32))
    return out
